# revision 1
# baseline (speedup 1.0000x reference)
"""Trainium2 Bass kernel for CandidateAwareAggregation.

Math (per batch b):
    pi = interest @ W1[:D]; pc = cand @ W1[D:]
    hidden = tanh(pi[k] + pc[c] + b1)                    (K, C, D)
    score[k, c] = hidden . W2[:, 0]     (b2 dropped: a constant shift
                                         is invariant under softmax_k)
    attn = softmax_k(score)
    out[c] = sum_k attn[k, c] * (interest[k] . cand[c])

Sharding: pure data parallel over the batch dim across 8 NeuronCores;
the tiny MLP weights are replicated.  The host only reshapes/casts the
inputs into the device layout (feature dim d on the 128 SBUF
partitions, fp16); all FLOPs run on-device.

Per core (b_loc = 128 batches):
  1. DMA iT (d x [b,k]), cT (d x [b,c]) f16 (host pre-transposed);
     derive cT2, a pair-interleaved c-padded copy, on-device.
  2. Project with stationary W1 halves -> piT (d x [k,b]),
     pcT (d x [c,b]) f16 (21 matmuls).
  3. Per block of nb batches: one broadcast-AP tensor_add builds all
     K*C*nb pre-activations (d x [c,bi,k]); tanh in place (+b1 as the
     per-partition activation bias); tensor_scalar_mul by w2 in place;
     gpsimd partition_all_reduce contracts d; two casting DMAs
     redistribute scores to sc_sb[(b%2)*64 + c, b*K + k].
  4. Dot scores: one matmul per b-pair (stationary = cT2 slice) lands
     in the same (pair x [b,k]) layout.
  5. Tail: Exp in place (softmax max-shift skipped: |score| < ~6 is
     safe in f32), segmented k-reductions for numerator/denominator,
     reciprocal, multiply, two strided PE transposes (even/odd b),
     store.
"""

import sys

for _p in ("/opt/trn_rl_repo", "/opt/pypackages"):
    if _p not in sys.path:
        sys.path.insert(0, _p)

import numpy as np

import concourse.bacc as bacc
import concourse.bass as bass
import concourse.bass_isa as bass_isa
import concourse.tile as tile
from concourse import mybir
from concourse.bass_utils import run_bass_kernel_spmd

B, K, C, D = 1024, 32, 50, 128
CP = 64
NCORES = 8
B_LOC = B // NCORES

F32 = mybir.dt.float32
F16 = mybir.dt.float16
Tanh = mybir.ActivationFunctionType.Tanh
Exp = mybir.ActivationFunctionType.Exp
ADD = mybir.AluOpType.add


def _ap(base, off, dims):
    return bass.AP(
        tensor=base.tensor,
        offset=base.offset + off,
        ap=[list(base.ap[0])] + [[int(s), int(n)] for s, n in dims],
    )


def _row(base, off, dims):
    """Single-partition (partition 0) AP with custom free dims."""
    return bass.AP(
        tensor=base.tensor,
        offset=base.offset + off,
        ap=[[list(base.ap[0])[0], 1]] + [[int(s), int(n)] for s, n in dims],
    )


def build_nc(b_loc=B_LOC, nb=8):
    assert b_loc % nb == 0 and nb % 2 == 0
    nblk = b_loc // nb

    nc = bacc.Bacc("TRN2", target_bir_lowering=False, debug=False)

    iT_d = nc.dram_tensor("iT", (D, b_loc * K), F16, kind="ExternalInput")
    cT_d = nc.dram_tensor("cT", (D, b_loc * C), F16, kind="ExternalInput")
    wi_d = nc.dram_tensor("wi", (D, D), F16, kind="ExternalInput")
    wc_d = nc.dram_tensor("wc", (D, D), F16, kind="ExternalInput")
    b1_d = nc.dram_tensor("b1", (D,), F32, kind="ExternalInput")
    w2_d = nc.dram_tensor("w2", (D, 1), F32, kind="ExternalInput")
    o_d = nc.dram_tensor("out", (b_loc, C), F32, kind="ExternalOutput")
    ident_d = nc.inline_tensor(np.eye(128, dtype=np.float32), name="ident")

    with tile.TileContext(nc) as tc:
        with (
            tc.tile_pool(name="consts", bufs=1) as consts,
            tc.tile_pool(name="big", bufs=1) as big,
            tc.tile_pool(name="prep", bufs=2) as prep,
            tc.tile_pool(name="arp", bufs=1) as arp,
            tc.tile_pool(name="small", bufs=1) as small,
            tc.tile_pool(name="tpsum", bufs=2, space="PSUM") as tpsum,
            tc.tile_pool(name="mpsum", bufs=2, space="PSUM") as mpsum,
            tc.tile_pool(name="dtps", bufs=2, space="PSUM") as dtps,
        ):
            ident = consts.tile([128, 128], F32, tag="ident")
            nc.sync.dma_start(out=ident[:], in_=ident_d[:])
            wi = consts.tile([128, 128], F16, tag="wi")
            nc.sync.dma_start(out=wi[:], in_=wi_d[:])
            wc = consts.tile([128, 128], F16, tag="wc")
            nc.sync.dma_start(out=wc[:], in_=wc_d[:])
            b1sb = consts.tile([128, 1], F32, tag="b1")
            nc.sync.dma_start(out=b1sb[:], in_=b1_d[:])
            w2sb = consts.tile([128, 1], F32, tag="w2")
            nc.sync.dma_start(out=w2sb[:], in_=w2_d[:])

            iT = big.tile([128, b_loc * K], F16, tag="iT")
            nc.sync.dma_start(out=iT[:], in_=iT_d[:])
            cT = big.tile([128, b_loc * C], F16, tag="cT")
            nc.sync.dma_start(out=cT[:], in_=cT_d[:])
            # pair-interleaved padded layout for the dot stationaries,
            # derived on-device: cT2 col = (b//2)*128 + (b%2)*64 + c
            cT2 = big.tile([128, b_loc * CP], F16, tag="cT2")
            nc.vector.memset(cT2[:], 0)
            nc.sync.dma_start(
                out=_ap(cT2[:], 0, [[128, b_loc // 2], [CP, 2], [1, C]]),
                in_=_ap(cT[:], 0, [[2 * C, b_loc // 2], [C, 2], [1, C]]),
            )

            piT = big.tile([128, K * b_loc], F16, tag="piT")
            pcT = big.tile([128, C * b_loc], F16, tag="pcT")

            def project(dst, w_st, srcT, n_items):
                per = max(1, 512 // b_loc)
                for j0 in range(0, n_items, per):
                    jn = min(per, n_items - j0)
                    ps = mpsum.tile([128, 512], F32, tag="mp")
                    rhs = _ap(srcT[:], j0, [[1, jn], [n_items, b_loc]])
                    nc.tensor.matmul(
                        ps[:, 0 : jn * b_loc], w_st[:], rhs, start=True, stop=True
                    )
                    nc.vector.tensor_copy(
                        out=dst[:, j0 * b_loc : (j0 + jn) * b_loc],
                        in_=ps[:, 0 : jn * b_loc],
                    )

            project(piT, wi, iT, K)
            project(pcT, wc, cT, C)

            # sc/dot layout: element (p, b*K + k), pair p = (b%2)*CP + c
            sc_sb = big.tile([128, b_loc * K], F32, tag="sc")
            dot_sb = big.tile([128, b_loc * K], F32, tag="dot")
            # initialize bands the redistribution DMAs never touch
            nc.vector.memset(sc_sb[:], 0)

            FD = K * C * nb  # pre free size per block
            for blk in range(nblk):
                b0 = blk * nb
                # a) pre col = c*(nb*K) + bi*K + k
                pre = prep.tile([128, FD], F16, tag="pre")
                nc.vector.tensor_add(
                    _ap(pre[:], 0, [[nb * K, C], [K, nb], [1, K]]),
                    _ap(piT[:], b0, [[0, C], [1, nb], [b_loc, K]]),
                    _ap(pcT[:], b0, [[b_loc, C], [1, nb], [0, K]]),
                )
                # b) tanh in place (contiguous), bias b1
                nc.scalar.activation(
                    out=pre[:], in_=pre[:], func=Tanh, bias=b1sb[:], scale=1.0
                )
                # c) w2 * hidden, in place (per-partition scalar)
                nc.vector.tensor_scalar_mul(pre[:], pre[:], w2sb[:])
                # d) partition reduce over d.  partition_all_reduce's APs are
                # invisible to Tile's dep tracker, so run the gpsimd sequence
                # inside a critical section bracketed by tracked ops: a touch
                # read of pre (waits for c) before, and gpsimd-initiated
                # casting DMAs (tracked writers of sc_sb) after, relying on
                # gpsimd FIFO order within the critical block.
                # e) redistribute scores: ar[0, c*(nb*K)+bi*K+k]
                #    -> sc_sb[(bi%2)*CP + c, (b0+bi)*K + k]
                # partition_all_reduce's APs are invisible to Tile's dep
                # tracker; rely on gpsimd FIFO order with tracked touch ops
                # before (waits for the DVE writes of pre) and after (marks
                # pre still-in-use until the reduce is done).
                ar = arp.tile([128, FD], F16, tag="ar")
                junk = arp.tile([1, 1], F32, tag="junk")
                nc.gpsimd.tensor_copy(out=ar[0:1, 0:1], in_=pre[0:1, 0:1])
                nc.gpsimd.partition_all_reduce(
                    ar[:], pre[:], channels=128, reduce_op=bass_isa.ReduceOp.add
                )
                nc.gpsimd.tensor_copy(out=junk[:], in_=pre[0:1, 0:1])
                for h in range(2):
                    nc.gpsimd.dma_start(
                        out=_ap(
                            sc_sb[h * CP : h * CP + C],
                            (b0 + h) * K,
                            [[2 * K, nb // 2], [1, K]],
                        ),
                        in_=_row(
                            ar[:],
                            h * K,
                            [[nb * K, C], [2 * K, nb // 2], [1, K]],
                        ),
                    )

            # dot scores: one matmul per b-pair
            for blk in range(nblk):
                b0 = blk * nb
                dt_ps = dtps.tile([128, nb * K], F32, tag="dtp")
                for j in range(nb // 2):
                    b = b0 + 2 * j
                    nc.tensor.matmul(
                        dt_ps[:, j * 2 * K : (j + 1) * 2 * K],
                        cT2[:, (b // 2) * 128 : (b // 2) * 128 + 128],
                        iT[:, b * K : (b + 2) * K],
                        start=True,
                        stop=True,
                    )
                nc.vector.tensor_copy(
                    out=dot_sb[:, b0 * K : (b0 + nb) * K], in_=dt_ps[:]
                )

            # ---------------- tail ----------------
            nc.scalar.activation(out=sc_sb[:], in_=sc_sb[:], func=Exp)
            den = small.tile([128, b_loc], F32, tag="den")
            nc.vector.tensor_reduce(
                out=den[:],
                in_=_ap(sc_sb[:], 0, [[K, b_loc], [1, K]]),
                axis=mybir.AxisListType.X,
                op=ADD,
            )
            nc.vector.tensor_mul(dot_sb[:], sc_sb[:], dot_sb[:])
            num = small.tile([128, b_loc], F32, tag="num")
            nc.vector.tensor_reduce(
                out=num[:],
                in_=_ap(dot_sb[:], 0, [[K, b_loc], [1, K]]),
                axis=mybir.AxisListType.X,
                op=ADD,
            )
            rec = small.tile([128, b_loc], F32, tag="rec")
            nc.vector.reciprocal(out=rec[:], in_=den[:])
            fin = small.tile([128, b_loc], F32, tag="fin")
            nc.vector.tensor_mul(fin[:], num[:], rec[:])

            # two strided transposes: even/odd b columns
            nbb = b_loc // 2
            for h in range(2):
                fp = tpsum.tile([128, 128], F32, tag="tp")
                nc.tensor.transpose(
                    fp[0:nbb, :], _ap(fin[:], h, [[2, nbb]]), ident[:]
                )
                osb = small.tile([128, C], F32, tag=f"osb{h}")
                nc.vector.tensor_copy(
                    out=osb[0:nbb, :], in_=fp[0:nbb, h * CP : h * CP + C]
                )
                o_flat = o_d[:].flatten_outer_dims()
                dst = bass.AP(
                    tensor=o_flat.tensor,
                    offset=o_flat.offset + h * C,
                    ap=[[2 * C, nbb], [1, C]],
                )
                nc.sync.dma_start(out=dst, in_=osb[0:nbb, :])

    nc.compile()
    return nc


_NC_CACHE = {}


def _get_nc(b_loc, nb=8):
    key = (b_loc, nb)
    if key not in _NC_CACHE:
        _NC_CACHE[key] = build_nc(b_loc, nb)
    return _NC_CACHE[key]


def make_in_maps(interest_vectors, candidate_vecs, W1, b1, W2, b_loc):
    W1 = np.asarray(W1, dtype=np.float32)
    wi = np.ascontiguousarray(W1[:D], dtype=np.float16)
    wc = np.ascontiguousarray(W1[D:], dtype=np.float16)
    b1 = np.ascontiguousarray(np.asarray(b1).reshape(D), dtype=np.float32)
    w2 = np.ascontiguousarray(
        np.asarray(W2, dtype=np.float32).reshape(D, 1)
    )

    ncores = interest_vectors.shape[0] // b_loc
    in_maps = []
    for i in range(ncores):
        s = slice(i * b_loc, (i + 1) * b_loc)
        iT = np.ascontiguousarray(
            np.asarray(interest_vectors[s], dtype=np.float16).reshape(b_loc * K, D).T
        )
        cv = np.asarray(candidate_vecs[s], dtype=np.float16)
        cT = np.ascontiguousarray(cv.reshape(b_loc * C, D).T)
        in_maps.append(
            {"iT": iT, "cT": cT, "wi": wi, "wc": wc, "b1": b1, "w2": w2}
        )
    return in_maps


def kernel(interest_vectors, candidate_vecs, W1, b1, W2, b2=None, **_ignored):
    interest_vectors = np.asarray(interest_vectors)
    candidate_vecs = np.asarray(candidate_vecs)
    b_tot = interest_vectors.shape[0]
    b_loc = b_tot // NCORES
    nc = _get_nc(b_loc)
    in_maps = make_in_maps(interest_vectors, candidate_vecs, W1, b1, W2, b_loc)
    res = run_bass_kernel_spmd(nc, in_maps, list(range(NCORES)))
    return np.concatenate([res.results[i]["out"] for i in range(NCORES)], axis=0)

